# revision 20
# baseline (speedup 1.0000x reference)
"""Distributed Trainium2 Bass kernel for AGFMAttentionDPLM2.

Sharding: 8 cores = 2 (batch) x 4 (head groups of 4 heads).
Per core: LayerNorm (modulate folded into effective QKV weights host-side),
QKV projections in transposed layout, RoPE via permutation-matmul, attention
in scoresT[k,q] orientation (softmax denominator from a ones-column in v),
AllGather of ctxT over the 4-core batch group, column-sharded output
projection with gate/residual epilogue.
"""
import sys

for _p in ("/opt/trn_rl_repo", "/opt/pypackages"):
    if _p not in sys.path:
        sys.path.append(_p)

import numpy as np
import ml_dtypes

import concourse.bass as bass
import concourse.mybir as mybir
import concourse.tile as tile
from concourse import bacc
from concourse.bass_utils import run_bass_kernel_spmd

B, S, H, NH, HD = 2, 2048, 1024, 16, 64
EPS = 1e-5
NCORES = 8
GPC = 4            # cores per batch group (head-tensor-parallel degree)
HPG = NH // GPC    # heads per core = 4
OC = HPG * HD      # feature cols per core = 256
HALF = S // 2

F32 = mybir.dt.float32
BF16 = mybir.dt.bfloat16

_CACHE = {}


def _rope_tables():
    inv_freq = 1.0 / (10000.0 ** (np.arange(0, HD, 2, dtype=np.float64) / HD))
    t = np.arange(HALF, dtype=np.float64)
    freqs = np.outer(t, inv_freq)                       # [1024, 32]
    emb = np.concatenate([freqs, freqs], axis=-1)       # [1024, 64]
    cos = np.cos(emb).T                                 # [64, 2048->]
    sin = np.sin(emb).T
    cos_full = np.concatenate([cos, cos], axis=1)       # [64, 2048]
    sin_full = np.concatenate([sin, sin], axis=1)
    cos2 = np.concatenate([cos_full, cos_full], axis=0)  # [128, 2048] pair-stacked
    sin2 = np.concatenate([sin_full, sin_full], axis=0)
    return cos2.astype(ml_dtypes.bfloat16), sin2.astype(ml_dtypes.bfloat16)


def _perm_matrix():
    # lhsT for rot = perm.T @ q : rot[d] = -q[d+32] (d%64<32), +q[d-32] otherwise
    p = np.zeros((128, 128), np.float32)
    for blk in range(2):
        o = blk * 64
        for d in range(32):
            p[o + 32 + d, o + d] = -1.0
            p[o + d, o + 32 + d] = 1.0
    return p.astype(ml_dtypes.bfloat16)


def _build_core_inputs(inputs):
    """Host-side shard + fold. Returns list of 8 dicts of numpy arrays."""
    f32 = np.float32
    hs = np.asarray(inputs["hidden_states"], f32)
    Wq, bq = np.asarray(inputs["Wq"], f32), np.asarray(inputs["bq"], f32)
    Wk, bk = np.asarray(inputs["Wk"], f32), np.asarray(inputs["bk"], f32)
    Wv, bv = np.asarray(inputs["Wv"], f32), np.asarray(inputs["bv"], f32)
    Wo, bo = np.asarray(inputs["Wo"], f32), np.asarray(inputs["bo"], f32)
    ln_g, ln_b = np.asarray(inputs["ln_g"], f32), np.asarray(inputs["ln_b"], f32)
    shift = [np.asarray(inputs["shift1"], f32), np.asarray(inputs["shift2"], f32)]
    scale = [np.asarray(inputs["scale1"], f32), np.asarray(inputs["scale2"], f32)]
    gate = [np.asarray(inputs["gate1"], f32), np.asarray(inputs["gate2"], f32)]

    cos2, sin2 = _rope_tables()
    perm = _perm_matrix()
    sel2 = np.zeros((2, 2, 128), np.float32)
    sel2[0, 0, :] = 1.0
    sel2[1, 1, :] = 1.0

    qscale = HD ** -0.5
    maps = []
    for c in range(NCORES):
        b, g = c // GPC, c % GPC
        cols = slice(g * OC, (g + 1) * OC)      # head cols for QKV; out cols for Wo
        m = {"hid": np.ascontiguousarray(hs[b]),
             "cos2": cos2, "sin2": sin2, "perm": perm, "sel2": sel2}

        # effective weights per half: x_mod = xn*A_i + C_i with
        # A_i = ln_g*(1+scale_i), C_i = ln_b*(1+scale_i)+shift_i   (per batch b)
        bqk = np.zeros((128, 8), f32)
        bvv = np.zeros((2, OC), f32)
        for hf in range(2):
            A = ln_g * (1.0 + scale[hf][b])          # [H]
            C = ln_b * (1.0 + scale[hf][b]) + shift[hf][b]
            for pi, (W, bias, pref) in enumerate(
                    ((Wq, bq, "wq"), (Wk, bk, "wk"), (Wv, bv, "wv"))):
                Wc = W[cols, :]                       # [256, H]
                Weff = (Wc * A[None, :]).T            # [H, 256]
                beff = Wc @ C + bias[cols]            # [256]
                if pref == "wq":
                    Weff = Weff * qscale
                    beff = beff * qscale
                m[f"{pref}{hf}"] = np.ascontiguousarray(Weff).astype(ml_dtypes.bfloat16)
                if pref == "wv":
                    bvv[hf] = beff
                else:
                    for pair in range(2):
                        bqk[:, pi * 4 + hf * 2 + pair] = beff[pair * 128:(pair + 1) * 128]
        m["bqk"] = bqk
        m["bvv"] = bvv

        m["wo"] = np.ascontiguousarray(Wo[cols, :].T).astype(ml_dtypes.bfloat16)  # [H, 256]
        # epilogue: out = gate_i*h + hidden, h = ctx@Wo^T + bo
        #   => out = gate_i*h_mm + (gate_i*bo + hidden)   (precompute residual)
        resid = np.ascontiguousarray(hs[b][:, cols]).copy()
        resid[:HALF] += gate[0][b][cols] * bo[cols]
        resid[HALF:] += gate[1][b][cols] * bo[cols]
        m["resid"] = resid
        m["gates"] = np.stack([gate[0][b][cols], gate[1][b][cols]])  # [2, 256]
        maps.append(m)
    return maps


def _build_graph(debug=False):
    nc = bacc.Bacc(None, target_bir_lowering=False, debug=False, num_devices=NCORES)

    # ---- external parameters (per-core shards) ----
    P = {}
    P["hid"] = nc.declare_dram_parameter("hid", [S, H], F32, isOutput=False)
    for hf in range(2):
        for pref in ("wq", "wk", "wv"):
            P[f"{pref}{hf}"] = nc.declare_dram_parameter(f"{pref}{hf}", [H, OC], BF16, isOutput=False)
    P["wo"] = nc.declare_dram_parameter("wo", [H, OC], BF16, isOutput=False)
    P["bqk"] = nc.declare_dram_parameter("bqk", [128, 8], F32, isOutput=False)
    P["bvv"] = nc.declare_dram_parameter("bvv", [2, OC], F32, isOutput=False)
    P["cos2"] = nc.declare_dram_parameter("cos2", [128, S], BF16, isOutput=False)
    P["sin2"] = nc.declare_dram_parameter("sin2", [128, S], BF16, isOutput=False)
    P["perm"] = nc.declare_dram_parameter("perm", [128, 128], BF16, isOutput=False)
    P["resid"] = nc.declare_dram_parameter("resid", [S, OC], F32, isOutput=False)
    P["gates"] = nc.declare_dram_parameter("gates", [2, OC], F32, isOutput=False)
    P["sel2"] = nc.declare_dram_parameter("sel2", [2, 2, 128], F32, isOutput=False)
    out_ext = nc.declare_dram_parameter("out", [S, OC], F32, isOutput=True)
    DBG = {}
    if debug:
        DBG["xnT"] = nc.declare_dram_parameter("d_xnT", [128, H // 128, S], BF16, isOutput=True)
        DBG["qT"] = nc.declare_dram_parameter("d_qT", [128, 2, S], BF16, isOutput=True)
        DBG["kT"] = nc.declare_dram_parameter("d_kT", [128, 2, S], BF16, isOutput=True)
        DBG["vaug"] = nc.declare_dram_parameter("d_vaug", [128, S // 128, NH // 4, HD + 2], BF16, isOutput=True)
        DBG["ctxU"] = nc.declare_dram_parameter("d_ctxU", [128, 2, S], BF16, isOutput=True)
        DBG["ctxF"] = nc.declare_dram_parameter("d_ctxF", [128, 8, S], BF16, isOutput=True)
        DBG["qb"] = nc.declare_dram_parameter("d_qb", [128, 512], BF16, isOutput=True)
        DBG["prot"] = nc.declare_dram_parameter("d_prot", [128, 512], F32, isOutput=True)
        DBG["t1"] = nc.declare_dram_parameter("d_t1", [128, 512], BF16, isOutput=True)
        DBG["t2"] = nc.declare_dram_parameter("d_t2", [128, 512], BF16, isOutput=True)
        DBG["bqk2"] = nc.declare_dram_parameter("d_bqk2", [128, 8], F32, isOutput=True)
        DBG["bvbc"] = nc.declare_dram_parameter("d_bvbc", [128, 2, OC], F32, isOutput=True)
        DBG["den"] = nc.declare_dram_parameter("d_den", [2, 1024], F32, isOutput=True)
        DBG["dtmp"] = nc.declare_dram_parameter("d_dtmp", [2, 1024], F32, isOutput=True)
        DBG["pb2"] = nc.declare_dram_parameter("d_pb2", [64, 512], F32, isOutput=True)

    # ---- internal DRAM for the collective ----
    cc_in = nc.dram_tensor("cc_in", [2, 128, S], BF16)
    cc_out = nc.dram_tensor("cc_out", [8, 128, S], BF16)

    NSC = S // 128           # 16 s-chunks
    NHC = H // 128           # 8 h-chunks

    with tile.TileContext(nc) as tc:
        with tc.tile_pool(name="persist", bufs=1) as pp:
            # persistent SBUF tensors
            xnT = pp.tile([128, NHC, S], BF16, name="xnT")
            qT = pp.tile([128, 2, S], BF16, name="qT")
            kT = pp.tile([128, 2, S], BF16, name="kT")
            v_aug = pp.tile([128, NSC, HPG, HD + 2], BF16, name="v_aug")
            ctxU = pp.tile([128, 2, S], BF16, name="ctxU")
            cos_sb = pp.tile([128, S], BF16, name="cos_sb")
            sin_sb = pp.tile([128, S], BF16, name="sin_sb")
            perm_sb = pp.tile([128, 128], BF16, name="perm_sb")
            bqk_sb = pp.tile([128, 8], F32, name="bqk_sb")
            bv_sb = pp.tile([2, OC], F32, name="bv_sb")
            g_sb = pp.tile([2, OC], F32, name="g_sb")
            sel2_sb = pp.tile([2, 2, 128], F32, name="sel2_sb")
            e0_sb = pp.tile([2, 64], BF16, name="e0_sb")
            gateb = pp.tile([128, 2, OC], F32, name="gateb")
            bv_bc = pp.tile([128, 2, OC], F32, name="bv_bc")
            wo_sb = pp.tile([128, NHC, OC], BF16, name="wo_sb")
            w_sb = {}
            for hf in range(2):
                for pref in ("wq", "wk", "wv"):
                    w_sb[(pref, hf)] = pp.tile([128, NHC, OC], BF16, name=f"{pref}{hf}_sb")

            nc.sync.dma_start(out=cos_sb[:], in_=P["cos2"][:])
            nc.sync.dma_start(out=sin_sb[:], in_=P["sin2"][:])
            nc.sync.dma_start(out=perm_sb[:], in_=P["perm"][:])
            nc.sync.dma_start(out=bqk_sb[:], in_=P["bqk"][:])
            nc.sync.dma_start(out=bv_sb[:], in_=P["bvv"][:])
            nc.sync.dma_start(out=g_sb[:], in_=P["gates"][:])
            nc.sync.dma_start(out=sel2_sb[:], in_=P["sel2"][:])
            for hf in range(2):
                for pref in ("wq", "wk", "wv"):
                    nc.sync.dma_start(
                        out=w_sb[(pref, hf)][:],
                        in_=P[f"{pref}{hf}"].rearrange("(c p) o -> p c o", p=128))
            nc.sync.dma_start(out=wo_sb[:], in_=P["wo"].rearrange("(c p) o -> p c o", p=128))
            nc.gpsimd.memset(e0_sb[:], 0.0)
            nc.gpsimd.memset(e0_sb[0:1, :], 1.0)
            nc.gpsimd.memset(v_aug[:, :, :, HD:HD + 2], 1.0)

            # gate broadcast [128, OC] per half via K=1 matmul
            with tc.tile_pool(name="ps_misc", bufs=2, space="PSUM") as ps_misc:
                for hf in range(2):
                    pg = ps_misc.tile([128, OC], F32, name="pg", tag="pg")
                    nc.tensor.matmul(pg[:], sel2_sb[:, hf, :], g_sb[:],
                                     start=True, stop=True)
                    nc.vector.tensor_copy(gateb[:, hf, :], pg[:])
                    pb = ps_misc.tile([128, OC], F32, name="pb", tag="pg")
                    nc.tensor.matmul(pb[:], sel2_sb[:, hf, :], bv_sb[:],
                                     start=True, stop=True)
                    nc.vector.tensor_copy(bv_bc[:, hf, :], pb[:])

            if True:
                # ---- Phase A: LayerNorm -> xn (bf16) -> transpose to xnT ----
                with tc.tile_pool(name="lnp", bufs=3) as lnp, \
                     tc.tile_pool(name="stat", bufs=4) as stp:
                    for sc in range(NSC):
                        hidt = lnp.tile([128, H], F32, name="hidt")
                        nc.sync.dma_start(out=hidt[:], in_=P["hid"][sc * 128:(sc + 1) * 128, :])
                        sq = lnp.tile([128, H], F32, name="sq", tag="sq")
                        ssum = stp.tile([128, 1], F32, name="ssum", tag="st")
                        ssq = stp.tile([128, 1], F32, name="ssq", tag="st")
                        nc.vector.tensor_tensor(sq[:], hidt[:], hidt[:], mybir.AluOpType.mult)
                        nc.vector.tensor_reduce(ssum[:], hidt[:], axis=mybir.AxisListType.X,
                                                op=mybir.AluOpType.add)
                        nc.vector.tensor_reduce(ssq[:], sq[:], axis=mybir.AxisListType.X,
                                                op=mybir.AluOpType.add)
                        mu = stp.tile([128, 1], F32, name="mu", tag="st")
                        var = stp.tile([128, 1], F32, name="var", tag="st")
                        rstd = stp.tile([128, 1], F32, name="rstd", tag="st")
                        nc.vector.tensor_scalar_mul(mu[:], ssum[:], 1.0 / H)
                        # var = ssq/H - mu^2 ; then rstd = sqrt(1/(var+eps))
                        nc.vector.tensor_tensor(var[:], mu[:], mu[:], mybir.AluOpType.mult)
                        nc.vector.scalar_tensor_tensor(
                            var[:], ssq[:], 1.0 / H, var[:],
                            op0=mybir.AluOpType.mult, op1=mybir.AluOpType.subtract)
                        nc.vector.tensor_scalar_add(var[:], var[:], EPS)
                        nc.vector.reciprocal(rstd[:], var[:])
                        nc.scalar.activation(rstd[:], rstd[:], mybir.ActivationFunctionType.Sqrt)
                        xn = lnp.tile([128, H], BF16, name="xn", tag="xn")
                        nc.vector.tensor_scalar(xn[:], hidt[:], mu[:], rstd[:],
                                                mybir.AluOpType.subtract, mybir.AluOpType.mult)
                        for hc in range(NHC):
                            nc.sync.dma_start_transpose(
                                xnT[:, hc, sc * 128:(sc + 1) * 128],
                                xn[:, hc * 128:(hc + 1) * 128])

                # ---- Phase B: QKV projections + RoPE ----
                with tc.tile_pool(name="ps_qk", bufs=2, space="PSUM") as ps_qk, \
                     tc.tile_pool(name="ps_rot", bufs=2, space="PSUM") as ps_rot, \
                     tc.tile_pool(name="ps_v", bufs=2, space="PSUM") as ps_v, \
                     tc.tile_pool(name="ropep", bufs=3) as rp:
                    for pi, (pref, dstT) in enumerate((("wq", qT), ("wk", kT))):
                        for hf in range(2):
                            for pair in range(2):
                                for w in range(2):
                                    off = hf * HALF + w * 512
                                    pq = ps_qk.tile([128, 512], F32, name="pq")
                                    for hc in range(NHC):
                                        nc.tensor.matmul(
                                            pq[:],
                                            w_sb[(pref, hf)][:, hc, pair * 128:(pair + 1) * 128],
                                            xnT[:, hc, off:off + 512],
                                            start=(hc == 0), stop=(hc == NHC - 1))
                                    q_b = rp.tile([128, 512], BF16, name="q_b", tag="q_b")
                                    nc.vector.tensor_scalar_add(
                                        q_b[:], pq[:], bqk_sb[:, pi * 4 + hf * 2 + pair:pi * 4 + hf * 2 + pair + 1])
                                    prot = ps_rot.tile([128, 512], F32, name="prot")
                                    nc.tensor.matmul(prot[:], perm_sb[:], q_b[:],
                                                     start=True, stop=True)
                                    t1 = rp.tile([128, 512], BF16, name="t1", tag="t1")
                                    nc.vector.tensor_tensor(t1[:], q_b[:], cos_sb[:, off:off + 512],
                                                            mybir.AluOpType.mult)
                                    t2 = rp.tile([128, 512], BF16, name="t2", tag="t2")
                                    nc.vector.tensor_tensor(t2[:], prot[:], sin_sb[:, off:off + 512],
                                                            mybir.AluOpType.mult)
                                    nc.vector.tensor_tensor(dstT[:, pair, off:off + 512],
                                                            t1[:], t2[:], mybir.AluOpType.add)
                                    if debug and pi == 0 and hf == 0 and pair == 0 and w == 0:
                                        nc.sync.dma_start(out=DBG["qb"][:], in_=q_b[:])
                                        vtmp = rp.tile([128, 512], F32, name="vtmp", tag="vtmp")
                                        nc.vector.tensor_copy(vtmp[:], prot[:])
                                        nc.sync.dma_start(out=DBG["prot"][:], in_=vtmp[:])
                                        nc.sync.dma_start(out=DBG["t1"][:], in_=t1[:])
                                        nc.sync.dma_start(out=DBG["t2"][:], in_=t2[:])
                    # v projection: [s-chunk, o] with bias broadcast
                    for sc in range(NSC):
                        hf = 0 if sc < NSC // 2 else 1
                        pv = ps_v.tile([128, OC], F32, name="pv")
                        for hc in range(NHC):
                            nc.tensor.matmul(pv[:], xnT[:, hc, sc * 128:(sc + 1) * 128],
                                             w_sb[("wv", hf)][:, hc, :],
                                             start=(hc == 0), stop=(hc == NHC - 1))
                        nc.vector.tensor_tensor(
                            v_aug[:, sc, :, 0:HD],
                            pv[:].rearrange("p (h d) -> p h d", h=HPG),
                            bv_bc[:, hf, :].rearrange("p (h d) -> p h d", h=HPG),
                            mybir.AluOpType.add)

                # ---- Phase C: attention per (head, q-window of 1024) ----
                NKT = NSC  # 16 k-tiles
                with tc.tile_pool(name="ps_s", bufs=2, space="PSUM") as ps_s, \
                     tc.tile_pool(name="ps_c", bufs=1, space="PSUM") as ps_c, \
                     tc.tile_pool(name="ps_b", bufs=2, space="PSUM") as ps_b, \
                     tc.tile_pool(name="expp", bufs=4) as ep, \
                     tc.tile_pool(name="denp", bufs=2) as dp:
                    for h in range(HPG):
                        prow = (h % 2) * 64
                        pidx = h // 2
                        for qw in range(2):
                            qoff = qw * 1024
                            pc = ps_c.tile([HD + 2, 1024], F32, name="pc")
                            for kt in range(NKT):
                                psn = ps_s.tile([128, 1024], F32, name="psn")
                                for w2 in range(2):
                                    nc.tensor.matmul(
                                        psn[:, w2 * 512:(w2 + 1) * 512],
                                        kT[prow:prow + 64, pidx, kt * 128:(kt + 1) * 128],
                                        qT[prow:prow + 64, pidx, qoff + w2 * 512:qoff + (w2 + 1) * 512],
                                        start=True, stop=True)
                                ex = ep.tile([128, 1024], BF16, name="ex", tag="ex")
                                nc.scalar.activation(ex[:], psn[:], mybir.ActivationFunctionType.Exp)
                                for w2 in range(2):
                                    nc.tensor.matmul(
                                        pc[:, w2 * 512:(w2 + 1) * 512],
                                        v_aug[:, kt, h, :],
                                        ex[:, w2 * 512:(w2 + 1) * 512],
                                        start=(kt == 0), stop=(kt == NKT - 1))
                            # softmax denominators -> reciprocal -> broadcast -> scale
                            dsb = dp.tile([2, 1024], F32, name="dsb", tag="dsb")
                            nc.vector.tensor_copy(dsb[:], pc[HD:HD + 2, :])
                            dtmp = dp.tile([2, 1024], F32, name="dtmp", tag="dtmp")
                            nc.vector.reciprocal_approx_fast(dtmp[:], dsb[:])
                            dtmpb = dp.tile([2, 1024], BF16, name="dtmpb", tag="dtmpb")
                            nc.vector.tensor_copy(dtmpb[:], dtmp[:])
                            if debug and h == 0 and qw == 0:
                                dcp = dp.tile([2, 1024], F32, name="dcp", tag="dcp")
                                nc.vector.tensor_copy(dcp[:], pc[HD:HD + 2, :])
                                nc.sync.dma_start(out=DBG["den"][:], in_=dcp[:])
                                nc.sync.dma_start(out=DBG["dtmp"][:], in_=dtmp[:])
                            nc.vector.tensor_copy(ctxU[prow:prow + 64, pidx, qoff:qoff + 1024],
                                                  pc[0:HD, :])
                            for w2 in range(2):
                                off = qoff + w2 * 512
                                pb2 = ps_b.tile([64, 512], F32, name="pb2")
                                nc.tensor.matmul(pb2[:], e0_sb[:],
                                                 dtmpb[:, w2 * 512:(w2 + 1) * 512],
                                                 start=True, stop=True)
                                if debug and h == 0 and qw == 0 and w2 == 0:
                                    pcp = dp.tile([64, 512], F32, name="pcp", tag="pcp")
                                    nc.vector.tensor_copy(pcp[:], pb2[:])
                                    nc.sync.dma_start(out=DBG["pb2"][:], in_=pcp[:])
                                nc.vector.tensor_tensor(
                                    ctxU[prow:prow + 64, pidx, off:off + 512],
                                    ctxU[prow:prow + 64, pidx, off:off + 512],
                                    pb2[:], mybir.AluOpType.mult)

                # ---- AllGather ctxT over the 4-core batch group ----
                for cchunk in range(2):
                    nc.sync.dma_start(out=cc_in[cchunk], in_=ctxU[:, cchunk, :])
                nc.gpsimd.collective_compute(
                    "AllGather", mybir.AluOpType.bypass,
                    replica_groups=[[0, 1, 2, 3], [4, 5, 6, 7]],
                    ins=[cc_in[:]], outs=[cc_out[:]])

            if debug:
                nc.sync.dma_start(out=DBG["bqk2"][:], in_=bqk_sb[:])
                nc.sync.dma_start(out=DBG["bvbc"][:], in_=bv_bc[:])
                nc.sync.dma_start(out=DBG["xnT"][:], in_=xnT[:])
                nc.sync.dma_start(out=DBG["qT"][:], in_=qT[:])
                nc.sync.dma_start(out=DBG["kT"][:], in_=kT[:])
                nc.sync.dma_start(out=DBG["vaug"][:], in_=v_aug[:])
                nc.sync.dma_start(out=DBG["ctxU"][:], in_=ctxU[:])

            # ---- Phase D: output projection + epilogue ----
            with tc.tile_pool(name="ctxf", bufs=1) as cfp, \
                 tc.tile_pool(name="ps_o", bufs=2, space="PSUM") as ps_o, \
                 tc.tile_pool(name="outp", bufs=3) as op:
                ctxF = cfp.tile([128, 8, S], BF16, name="ctxF")
                nc.sync.dma_start(out=ctxF[:], in_=cc_out.rearrange("c p s -> p c s"))
                if debug:
                    nc.sync.dma_start(out=DBG["ctxF"][:], in_=ctxF[:])
                for sc in range(NSC):
                    hf = 0 if sc < NSC // 2 else 1
                    po = ps_o.tile([128, OC], F32, name="po")
                    for cc in range(8):
                        nc.tensor.matmul(po[:], ctxF[:, cc, sc * 128:(sc + 1) * 128],
                                         wo_sb[:, cc, :],
                                         start=(cc == 0), stop=(cc == 7))
                    rs = op.tile([128, OC], F32, name="rs", tag="rs")
                    nc.sync.dma_start(out=rs[:], in_=P["resid"][sc * 128:(sc + 1) * 128, :])
                    t = op.tile([128, OC], F32, name="t", tag="t")
                    nc.vector.tensor_tensor(t[:], po[:], gateb[:, hf, :], mybir.AluOpType.mult)
                    ot = op.tile([128, OC], F32, name="ot", tag="ot")
                    nc.vector.tensor_tensor(ot[:], t[:], rs[:], mybir.AluOpType.add)
                    nc.sync.dma_start(out=out_ext[sc * 128:(sc + 1) * 128, :], in_=ot[:])

    nc.compile()
    return nc


def kernel(**inputs) -> np.ndarray:
    if "nc" not in _CACHE:
        _CACHE["nc"] = _build_graph()
    nc = _CACHE["nc"]
    in_maps = _build_core_inputs(inputs)
    res = run_bass_kernel_spmd(nc, in_maps, core_ids=list(range(NCORES))).results
    out = np.empty((B, S, H), np.float32)
    for c in range(NCORES):
        b, g = c // GPC, c % GPC
        out[b, :, g * OC:(g + 1) * OC] = res[c]["out"]
    return out
